# revision 2
# baseline (speedup 1.0000x reference)
"""GQA attention (B=2,T=2048,C=2048,H=16,KV=4,D=128) + RoPE + causal softmax
+ output projection, tensor-parallel over 8 NeuronCores (2 q-heads/core).

Contract: kernel(**inputs) takes full numpy inputs, returns full output.
Per-core partial outputs (o @ Wo[rows]) are summed on the host.
"""

import sys

sys.path.insert(0, "/opt/trn_rl_repo")

import numpy as np

import concourse.bacc as bacc
import concourse.mybir as mybir
import concourse.tile as tile
from concourse.bass_utils import run_bass_kernel_spmd

B, T, C = 2, 2048, 2048
H, KV, D = 16, 4, 128
G = H // KV
N_CORES = 8
HL = H // N_CORES  # 2 q-heads per core
BT = B * T  # 4096
NCH = BT // 512  # 8 token chunks of 512
KCH = C // 128  # 16 contraction chunks
QC = T // 512  # 4 q chunks per batch
KC = T // 128  # 16 k chunks per batch

F16 = mybir.dt.float16
F32 = mybir.dt.float32

_cache = {}


def _build_program():
    nc = bacc.Bacc("TRN2", target_bir_lowering=False, debug=False,
                   num_devices=N_CORES)

    xT = nc.dram_tensor("xT", [C, BT], F16, kind="ExternalInput").ap()
    wq = nc.dram_tensor("wq", [C, HL * D], F16, kind="ExternalInput").ap()
    wk = nc.dram_tensor("wk", [C, D], F16, kind="ExternalInput").ap()
    wv = nc.dram_tensor("wv", [C, D], F16, kind="ExternalInput").ap()
    wo = nc.dram_tensor("wo", [HL * D, C], F16, kind="ExternalInput").ap()
    cosq = nc.dram_tensor("cosq", [D, T], F32, kind="ExternalInput").ap()
    sinq = nc.dram_tensor("sinq", [D, T], F32, kind="ExternalInput").ap()
    cosk = nc.dram_tensor("cosk", [D, T], F32, kind="ExternalInput").ap()
    sink = nc.dram_tensor("sink", [D, T], F32, kind="ExternalInput").ap()
    masks = nc.dram_tensor("masks", [128, 4, 512], F16, kind="ExternalInput").ap()
    y = nc.dram_tensor("y", [BT, C], F32, kind="ExternalOutput").ap()

    with tile.TileContext(nc) as tc:
        with (
            tc.tile_pool(name="cpool", bufs=1) as cpool,
            tc.tile_pool(name="xpool", bufs=2) as xpool,
            tc.tile_pool(name="ppool", bufs=6) as ppool,
            tc.tile_pool(name="rpool", bufs=2) as rpool,
            tc.tile_pool(name="bpool", bufs=2) as bpool,
            tc.tile_pool(name="ypool", bufs=6) as ypool,
            tc.tile_pool(name="psA", bufs=2, space="PSUM") as psA,
            tc.tile_pool(name="psS", bufs=2, space="PSUM") as psS,
            tc.tile_pool(name="psV", bufs=1, space="PSUM") as psV,
            tc.tile_pool(name="psD", bufs=1, space="PSUM") as psD,
        ):
            # ---- persistent SBUF state ----
            wq_s = cpool.tile([128, KCH, HL * D], F16, name="wq_s")
            wk_s = cpool.tile([128, KCH, D], F16, name="wk_s")
            wv_s = cpool.tile([128, KCH, D], F16, name="wv_s")
            wo_s = cpool.tile([128, HL, C], F16, name="wo_s")
            cosq_s = cpool.tile([D, T], F32, name="cosq_s")
            sinq_s = cpool.tile([D, T], F32, name="sinq_s")
            cosk_s = cpool.tile([D, T], F32, name="cosk_s")
            sink_s = cpool.tile([D, T], F32, name="sink_s")
            masks_s = cpool.tile([128, 4, 512], F16, name="masks_s")
            ones_s = cpool.tile([128, 1], F16, name="ones_s")
            qT = cpool.tile([D, HL, BT], F16, name="qT")
            kT = cpool.tile([D, BT], F16, name="kT")
            vv = cpool.tile([128, BT // 128, D], F16, name="vv")
            oT = cpool.tile([D, HL, BT], F16, name="oT")

            nc.sync.dma_start(wq_s[:], wq.rearrange("(k p) m -> p k m", p=128))
            nc.sync.dma_start(wk_s[:], wk.rearrange("(k p) m -> p k m", p=128))
            nc.sync.dma_start(wv_s[:], wv.rearrange("(k p) m -> p k m", p=128))
            nc.sync.dma_start(wo_s[:], wo.rearrange("(r p) n -> p r n", p=128))
            nc.sync.dma_start(cosq_s[:], cosq[:])
            nc.sync.dma_start(sinq_s[:], sinq[:])
            nc.sync.dma_start(cosk_s[:], cosk[:])
            nc.sync.dma_start(sink_s[:], sink[:])
            nc.sync.dma_start(masks_s[:], masks[:])
            nc.vector.memset(ones_s[:], 1.0)

            xT_r = xT.rearrange("(k p) n -> p k n", p=128)

            def rope(dst, ps, cos_s, sin_s, t0):
                t1 = rpool.tile([128, 512], F32, name="t1")
                t2 = rpool.tile([128, 512], F32, name="t2")
                mult = mybir.AluOpType.mult
                nc.vector.tensor_tensor(t1[:], ps[:], cos_s[:, t0:t0 + 512], mult)
                nc.vector.tensor_tensor(t2[0:64, :], ps[64:128, :],
                                        sin_s[0:64, t0:t0 + 512], mult)
                nc.vector.tensor_tensor(t2[64:128, :], ps[0:64, :],
                                        sin_s[64:128, t0:t0 + 512], mult)
                nc.vector.tensor_tensor(dst, t1[:], t2[:], mybir.AluOpType.add)

            # ---- phase 1: QKV projections (token chunks of 512) ----
            for n in range(NCH):
                n0 = n * 512
                t0 = (n % QC) * 512  # rope-table offset (within batch)
                xt = xpool.tile([128, KCH, 512], F16, name="xt")
                nc.sync.dma_start(xt[:], xT_r[:, :, n0:n0 + 512])
                for h in range(HL):
                    psq = psA.tile([128, 512], F32, name="psq", tag="psA")
                    for kc in range(KCH):
                        nc.tensor.matmul(
                            psq[:],
                            lhsT=wq_s[:, kc, h * D:(h + 1) * D],
                            rhs=xt[:, kc, :],
                            start=(kc == 0), stop=(kc == KCH - 1))
                    rope(qT[:, h, n0:n0 + 512], psq, cosq_s, sinq_s, t0)
                psk = psA.tile([128, 512], F32, name="psk", tag="psA")
                for kc in range(KCH):
                    nc.tensor.matmul(psk[:], lhsT=wk_s[:, kc, :],
                                     rhs=xt[:, kc, :],
                                     start=(kc == 0), stop=(kc == KCH - 1))
                rope(kT[:, n0:n0 + 512], psk, cosk_s, sink_s, t0)
                for s in range(4):
                    psv = psA.tile([128, 512], F32, name="psv", tag="psA")
                    for kc in range(KCH):
                        nc.tensor.matmul(
                            psv[:, 0:D],
                            lhsT=xt[:, kc, s * 128:(s + 1) * 128],
                            rhs=wv_s[:, kc, :],
                            start=(kc == 0), stop=(kc == KCH - 1))
                    nc.vector.tensor_copy(vv[:, n * 4 + s, :], psv[:, 0:D])

            # ---- phase 2: attention per (b, h) ----
            mult = mybir.AluOpType.mult
            for b in range(B):
                base = b * T
                for h in range(HL):
                    for qc in range(QC):
                        q0 = base + qc * 512
                        nkc = 4 * (qc + 1)  # causal: k chunks 0..nkc-1
                        npair = nkc // 2
                        psv2 = psV.tile([128, 512], F32, name="psv2")
                        psd = psD.tile([128, 512], F32, name="psd")
                        pts = [None] * npair

                        def emit_scores(pi):
                            pss = psS.tile([128, 2, 512], F32, name="pss")
                            pt = ppool.tile([128, 2, 512], F16, name="pt")
                            for i in range(2):
                                kc = pi * 2 + i
                                nc.tensor.matmul(
                                    pss[:, i, :],
                                    lhsT=kT[:, base + kc * 128:base + (kc + 1) * 128],
                                    rhs=qT[:, h, q0:q0 + 512],
                                    start=True, stop=True)
                            nc.scalar.activation(
                                pt[:], pss[:], mybir.ActivationFunctionType.Exp)
                            d0 = pi * 2 - 4 * qc  # delta of first kc in pair
                            if d0 >= 0:  # straddle pair: causal mask
                                nc.vector.tensor_tensor(
                                    pt[:], pt[:], masks_s[:, d0:d0 + 2, :], mult)
                            pts[pi] = pt

                        def emit_consume(pi):
                            pt = pts[pi]
                            for i in range(2):
                                kc = pi * 2 + i
                                first = (kc == 0)
                                last = (kc == nkc - 1)
                                nc.tensor.matmul(
                                    psd[0:1, :], lhsT=ones_s[:],
                                    rhs=pt[:, i, :], start=first, stop=last)
                                nc.tensor.matmul(
                                    psv2[:], lhsT=vv[:, b * KC + kc, :],
                                    rhs=pt[:, i, :], start=first, stop=last)

                        # 1-stage software pipeline so PE's consume matmuls
                        # never sit immediately behind the exp they wait on
                        for pi in range(npair + 1):
                            if pi < npair:
                                emit_scores(pi)
                            if pi > 0:
                                emit_consume(pi - 1)

                        recip = rpool.tile([1, 512], F32, name="recip")
                        nc.vector.reciprocal(recip[:], psd[0:1, :])
                        bc = bpool.tile([128, 512], F32, name="bc")
                        nc.gpsimd.partition_broadcast(bc[:], recip[:])
                        nc.vector.tensor_tensor(
                            oT[:, h, q0:q0 + 512], psv2[:], bc[:], mult)

            # ---- phase 3: output projection ----
            idx = 0
            for nt in range(BT // 128):
                t0 = nt * 128
                for cc in range(C // 512):
                    c0 = cc * 512
                    pso = psA.tile([128, 512], F32, name="pso", tag="psA")
                    for h in range(HL):
                        nc.tensor.matmul(
                            pso[:], lhsT=oT[:, h, t0:t0 + 128],
                            rhs=wo_s[:, h, c0:c0 + 512],
                            start=(h == 0), stop=(h == HL - 1))
                    yt = ypool.tile([128, 512], F32, name="yt")
                    # balance PSUM eviction between ACT and DVE
                    if idx % 8 < 3:
                        nc.scalar.copy(yt[:], pso[:])
                    else:
                        nc.vector.tensor_copy(yt[:], pso[:])
                    idx += 1
                    nc.sync.dma_start(y[t0:t0 + 128, c0:c0 + 512], yt[:])

    nc.compile()
    return nc


def _rope_tables():
    inv = (1.0 / (10000.0 ** (np.arange(0, D, 2, dtype=np.float32) / D)))
    f = np.arange(T, dtype=np.float32)[:, None] * inv[None, :]  # [T, 64]
    cos = np.concatenate([np.cos(f)] * 2, axis=-1).astype(np.float32)  # [T,128]
    sin = np.concatenate([np.sin(f)] * 2, axis=-1).astype(np.float32)
    sgn = np.ones((D,), np.float32)
    sgn[0:64] = -1.0  # sign-folded for the rotate-half formulation
    sinf = sin * sgn[None, :]
    alpha = np.float32(1.0 / np.sqrt(D))
    return (cos.T.copy() * alpha, sinf.T.copy() * alpha,
            cos.T.copy(), sinf.T.copy())


def _masks():
    i = np.arange(128)[:, None]
    j = np.arange(512)[None, :]
    m = np.stack([(j >= i + 128 * d) for d in range(4)], axis=0)  # [4,128,512]
    return np.ascontiguousarray(m.transpose(1, 0, 2)).astype(np.float16)


def make_in_maps(x, Wq, Wk, Wv, Wo):
    xT = np.ascontiguousarray(
        x.reshape(BT, C).T).astype(np.float16)  # [C, BT]
    cq, sq, ck, sk = _rope_tables()
    mk = _masks()
    in_maps = []
    for c in range(N_CORES):
        g = c // 2  # kv head for this core's 2 q-heads
        in_maps.append({
            "xT": xT,
            "wq": np.ascontiguousarray(
                Wq[:, c * HL * D:(c + 1) * HL * D]).astype(np.float16),
            "wk": np.ascontiguousarray(
                Wk[:, g * D:(g + 1) * D]).astype(np.float16),
            "wv": np.ascontiguousarray(
                Wv[:, g * D:(g + 1) * D]).astype(np.float16),
            "wo": np.ascontiguousarray(
                Wo[c * HL * D:(c + 1) * HL * D, :]).astype(np.float16),
            "cosq": cq, "sinq": sq, "cosk": ck, "sink": sk,
            "masks": mk,
        })
    return in_maps


def get_program():
    if "nc" not in _cache:
        _cache["nc"] = _build_program()
    return _cache["nc"]


def kernel(x, Wq, Wk, Wv, Wo):
    nc = get_program()
    in_maps = make_in_maps(x, Wq, Wk, Wv, Wo)
    res = run_bass_kernel_spmd(nc, in_maps, core_ids=list(range(N_CORES)))
    out = np.zeros((BT, C), np.float32)
    for c in range(N_CORES):
        out += res.results[c]["y"]
    return out.reshape(B, T, C)


if __name__ == "__main__":
    rng = np.random.default_rng(0)
    x = rng.standard_normal((B, T, C), dtype=np.float32)
    Wq = rng.standard_normal((C, H * D), dtype=np.float32) * 0.02
    Wk = rng.standard_normal((C, KV * D), dtype=np.float32) * 0.02
    Wv = rng.standard_normal((C, KV * D), dtype=np.float32) * 0.02
    Wo = rng.standard_normal((C, C), dtype=np.float32) * 0.02
    out = kernel(x=x, Wq=Wq, Wk=Wk, Wv=Wv, Wo=Wo)
    print("out", out.shape, out.dtype, float(np.abs(out).max()))


# revision 5
# speedup vs baseline: 6.5158x; 6.5158x over previous
"""GQA attention (B=2,T=2048,C=2048,H=16,KV=4,D=128) + RoPE + causal softmax
+ output projection, tensor-parallel over 8 NeuronCores (2 q-heads/core).

Contract: kernel(**inputs) takes full numpy inputs, returns full output.
Per-core partial outputs (o @ Wo[rows]) are summed on the host.
"""

import sys

sys.path.insert(0, "/opt/trn_rl_repo")

import numpy as np

import concourse.bacc as bacc
import concourse.mybir as mybir
import concourse.tile as tile
from concourse.bass_utils import run_bass_kernel_spmd

B, T, C = 2, 2048, 2048
H, KV, D = 16, 4, 128
G = H // KV
N_CORES = 8
HL = H // N_CORES  # 2 q-heads per core
BT = B * T  # 4096
NCH = BT // 512  # 8 token chunks of 512
KCH = C // 128  # 16 contraction chunks
QC = T // 512  # 4 q chunks per batch
KC = T // 128  # 16 k chunks per batch

F16 = mybir.dt.float16
F32 = mybir.dt.float32

_cache = {}


class _St:
    pass


def _emit_body(nc, st):
    """One full forward pass. Emitted `reps` times for slope timing."""
    mult = mybir.AluOpType.mult

    def rope(dst, ps, cos_s, sin_s, t0):
        t1 = st.rpool.tile([128, 512], F32, name="t1")
        t2 = st.rpool.tile([128, 512], F32, name="t2")
        nc.vector.tensor_tensor(t1[:], ps[:], cos_s[:, t0:t0 + 512], mult)
        nc.vector.tensor_tensor(t2[0:64, :], ps[64:128, :],
                                sin_s[0:64, t0:t0 + 512], mult)
        nc.vector.tensor_tensor(t2[64:128, :], ps[0:64, :],
                                sin_s[64:128, t0:t0 + 512], mult)
        nc.vector.tensor_tensor(dst, t1[:], t2[:], mybir.AluOpType.add)

    # ---- phase 1: QKV projections (token chunks of 512) ----
    for n in range(NCH):
        n0 = n * 512
        t0 = (n % QC) * 512  # rope-table offset (within batch)
        xt = st.xpool.tile([128, KCH, 512], F16, name="xt")
        nc.sync.dma_start(xt[:], st.xT_r[:, :, n0:n0 + 512])
        for h in range(HL):
            psq = st.psA.tile([128, 512], F32, name="psq", tag="psA")
            for kc in range(KCH):
                nc.tensor.matmul(
                    psq[:],
                    lhsT=st.wq_s[:, kc, h * D:(h + 1) * D],
                    rhs=xt[:, kc, :],
                    start=(kc == 0), stop=(kc == KCH - 1))
            rope(st.qT[:, h, n0:n0 + 512], psq, st.cosq_s, st.sinq_s, t0)
        psk = st.psA.tile([128, 512], F32, name="psk", tag="psA")
        for kc in range(KCH):
            nc.tensor.matmul(psk[:], lhsT=st.wk_s[:, kc, :],
                             rhs=xt[:, kc, :],
                             start=(kc == 0), stop=(kc == KCH - 1))
        rope(st.kT[:, n0:n0 + 512], psk, st.cosk_s, st.sink_s, t0)
        for s in range(4):
            psv = st.psA.tile([128, 512], F32, name="psv", tag="psA")
            for kc in range(KCH):
                nc.tensor.matmul(
                    psv[:, 0:D],
                    lhsT=xt[:, kc, s * 128:(s + 1) * 128],
                    rhs=st.wv_s[:, kc, :],
                    start=(kc == 0), stop=(kc == KCH - 1))
            nc.vector.tensor_copy(st.vv[:, n * 4 + s, :], psv[:, 0:D])

    # ---- phase 2: attention per (b, h) ----
    for b in range(B):
        base = b * T
        for h in range(HL):
            for qc in range(QC):
                q0 = base + qc * 512
                nkc = 4 * (qc + 1)  # causal: k chunks 0..nkc-1
                npair = nkc // 2
                psv2 = st.psV.tile([128, 512], F32, name="psv2")
                psd = st.psD.tile([128, 512], F32, name="psd")
                pts = [None] * npair

                def emit_scores(pi):
                    pss = st.psS.tile([128, 2, 512], F32, name="pss")
                    pt = st.ppool.tile([128, 2, 512], F16, name="pt")
                    for i in range(2):
                        kc = pi * 2 + i
                        nc.tensor.matmul(
                            pss[:, i, :],
                            lhsT=st.kT[:, base + kc * 128:base + (kc + 1) * 128],
                            rhs=st.qT[:, h, q0:q0 + 512],
                            start=True, stop=True)
                    nc.scalar.activation(
                        pt[:], pss[:], mybir.ActivationFunctionType.Exp)
                    d0 = pi * 2 - 4 * qc  # delta of first kc in pair
                    if d0 >= 0:  # straddle pair: causal mask
                        nc.vector.tensor_tensor(
                            pt[:], pt[:], st.masks_s[:, d0:d0 + 2, :], mult)
                    pts[pi] = pt

                def emit_consume(pi):
                    pt = pts[pi]
                    for i in range(2):
                        kc = pi * 2 + i
                        first = (kc == 0)
                        last = (kc == nkc - 1)
                        nc.tensor.matmul(
                            psd[0:1, :], lhsT=st.ones_s[:],
                            rhs=pt[:, i, :], start=first, stop=last)
                        nc.tensor.matmul(
                            psv2[:], lhsT=st.vv[:, b * KC + kc, :],
                            rhs=pt[:, i, :], start=first, stop=last)

                # 1-stage software pipeline: PE's consume matmuls never sit
                # immediately behind the exp they wait on
                for pi in range(npair + 1):
                    if pi < npair:
                        emit_scores(pi)
                    if pi > 0:
                        emit_consume(pi - 1)

                recip = st.rpool.tile([1, 512], F32, name="recip")
                nc.vector.reciprocal(recip[:], psd[0:1, :])
                bc = st.bpool.tile([128, 512], F32, name="bc")
                nc.gpsimd.partition_broadcast(bc[:], recip[:])
                nc.vector.tensor_tensor(
                    st.oT[:, h, q0:q0 + 512], psv2[:], bc[:], mult)

    # ---- phase 3: output projection ----
    idx = 0
    for nt in range(BT // 128):
        t0 = nt * 128
        for cc in range(C // 512):
            c0 = cc * 512
            pso = st.psA.tile([128, 512], F32, name="pso", tag="psA")
            for h in range(HL):
                nc.tensor.matmul(
                    pso[:], lhsT=st.oT[:, h, t0:t0 + 128],
                    rhs=st.wo_s[:, h, c0:c0 + 512],
                    start=(h == 0), stop=(h == HL - 1))
            yt = st.ypool.tile([128, 512], F32, name="yt")
            # balance PSUM eviction between ACT and DVE
            if idx % 8 < 3:
                nc.scalar.copy(yt[:], pso[:])
            else:
                nc.vector.tensor_copy(yt[:], pso[:])
            idx += 1
            nc.sync.dma_start(st.y[t0:t0 + 128, c0:c0 + 512], yt[:])


def _build_program(reps=1):
    nc = bacc.Bacc("TRN2", target_bir_lowering=False, debug=False,
                   num_devices=N_CORES)

    xT = nc.dram_tensor("xT", [C, BT], F16, kind="ExternalInput").ap()
    wq = nc.dram_tensor("wq", [C, HL * D], F16, kind="ExternalInput").ap()
    wk = nc.dram_tensor("wk", [C, D], F16, kind="ExternalInput").ap()
    wv = nc.dram_tensor("wv", [C, D], F16, kind="ExternalInput").ap()
    wo = nc.dram_tensor("wo", [HL * D, C], F16, kind="ExternalInput").ap()
    cosq = nc.dram_tensor("cosq", [D, T], F32, kind="ExternalInput").ap()
    sinq = nc.dram_tensor("sinq", [D, T], F32, kind="ExternalInput").ap()
    cosk = nc.dram_tensor("cosk", [D, T], F32, kind="ExternalInput").ap()
    sink = nc.dram_tensor("sink", [D, T], F32, kind="ExternalInput").ap()
    masks = nc.dram_tensor("masks", [128, 4, 512], F16, kind="ExternalInput").ap()
    y = nc.dram_tensor("y", [BT, C], F32, kind="ExternalOutput").ap()

    st = _St()
    with tile.TileContext(nc) as tc:
        with (
            tc.tile_pool(name="cpool", bufs=1) as cpool,
            tc.tile_pool(name="xpool", bufs=2) as xpool,
            tc.tile_pool(name="ppool", bufs=6) as ppool,
            tc.tile_pool(name="rpool", bufs=2) as rpool,
            tc.tile_pool(name="bpool", bufs=2) as bpool,
            tc.tile_pool(name="ypool", bufs=6) as ypool,
            tc.tile_pool(name="psA", bufs=2, space="PSUM") as psA,
            tc.tile_pool(name="psS", bufs=2, space="PSUM") as psS,
            tc.tile_pool(name="psV", bufs=1, space="PSUM") as psV,
            tc.tile_pool(name="psD", bufs=1, space="PSUM") as psD,
        ):
            st.xpool, st.ppool, st.rpool, st.bpool, st.ypool = (
                xpool, ppool, rpool, bpool, ypool)
            st.psA, st.psS, st.psV, st.psD = psA, psS, psV, psD

            # ---- persistent SBUF state ----
            st.wq_s = cpool.tile([128, KCH, HL * D], F16, name="wq_s")
            st.wk_s = cpool.tile([128, KCH, D], F16, name="wk_s")
            st.wv_s = cpool.tile([128, KCH, D], F16, name="wv_s")
            st.wo_s = cpool.tile([128, HL, C], F16, name="wo_s")
            st.cosq_s = cpool.tile([D, T], F32, name="cosq_s")
            st.sinq_s = cpool.tile([D, T], F32, name="sinq_s")
            st.cosk_s = cpool.tile([D, T], F32, name="cosk_s")
            st.sink_s = cpool.tile([D, T], F32, name="sink_s")
            st.masks_s = cpool.tile([128, 4, 512], F16, name="masks_s")
            st.ones_s = cpool.tile([128, 1], F16, name="ones_s")
            st.qT = cpool.tile([D, HL, BT], F16, name="qT")
            st.kT = cpool.tile([D, BT], F16, name="kT")
            st.vv = cpool.tile([128, BT // 128, D], F16, name="vv")
            st.oT = cpool.tile([D, HL, BT], F16, name="oT")

            nc.sync.dma_start(st.wq_s[:], wq.rearrange("(k p) m -> p k m", p=128))
            nc.sync.dma_start(st.wk_s[:], wk.rearrange("(k p) m -> p k m", p=128))
            nc.sync.dma_start(st.wv_s[:], wv.rearrange("(k p) m -> p k m", p=128))
            nc.sync.dma_start(st.wo_s[:], wo.rearrange("(r p) n -> p r n", p=128))
            nc.sync.dma_start(st.cosq_s[:], cosq[:])
            nc.sync.dma_start(st.sinq_s[:], sinq[:])
            nc.sync.dma_start(st.cosk_s[:], cosk[:])
            nc.sync.dma_start(st.sink_s[:], sink[:])
            nc.sync.dma_start(st.masks_s[:], masks[:])
            nc.vector.memset(st.ones_s[:], 1.0)

            st.xT_r = xT.rearrange("(k p) n -> p k n", p=128)
            st.y = y

            for _rep in range(reps):
                _emit_body(nc, st)

    nc.compile()
    return nc


def _rope_tables():
    inv = (1.0 / (10000.0 ** (np.arange(0, D, 2, dtype=np.float32) / D)))
    f = np.arange(T, dtype=np.float32)[:, None] * inv[None, :]  # [T, 64]
    cos = np.concatenate([np.cos(f)] * 2, axis=-1).astype(np.float32)  # [T,128]
    sin = np.concatenate([np.sin(f)] * 2, axis=-1).astype(np.float32)
    sgn = np.ones((D,), np.float32)
    sgn[0:64] = -1.0  # sign-folded for the rotate-half formulation
    sinf = sin * sgn[None, :]
    alpha = np.float32(1.0 / np.sqrt(D))
    return (cos.T.copy() * alpha, sinf.T.copy() * alpha,
            cos.T.copy(), sinf.T.copy())


def _masks():
    i = np.arange(128)[:, None]
    j = np.arange(512)[None, :]
    m = np.stack([(j >= i + 128 * d) for d in range(4)], axis=0)  # [4,128,512]
    return np.ascontiguousarray(m.transpose(1, 0, 2)).astype(np.float16)


def make_in_maps(x, Wq, Wk, Wv, Wo):
    xT = np.ascontiguousarray(
        x.reshape(BT, C).T).astype(np.float16)  # [C, BT]
    cq, sq, ck, sk = _rope_tables()
    mk = _masks()
    in_maps = []
    for c in range(N_CORES):
        g = c // 2  # kv head for this core's 2 q-heads
        in_maps.append({
            "xT": xT,
            "wq": np.ascontiguousarray(
                Wq[:, c * HL * D:(c + 1) * HL * D]).astype(np.float16),
            "wk": np.ascontiguousarray(
                Wk[:, g * D:(g + 1) * D]).astype(np.float16),
            "wv": np.ascontiguousarray(
                Wv[:, g * D:(g + 1) * D]).astype(np.float16),
            "wo": np.ascontiguousarray(
                Wo[c * HL * D:(c + 1) * HL * D, :]).astype(np.float16),
            "cosq": cq, "sinq": sq, "cosk": ck, "sink": sk,
            "masks": mk,
        })
    return in_maps


def get_program(reps=1):
    key = ("nc", reps)
    if key not in _cache:
        _cache[key] = _build_program(reps)
    return _cache[key]


def kernel(x, Wq, Wk, Wv, Wo):
    nc = get_program()
    in_maps = make_in_maps(x, Wq, Wk, Wv, Wo)
    res = run_bass_kernel_spmd(nc, in_maps, core_ids=list(range(N_CORES)))
    out = np.zeros((BT, C), np.float32)
    for c in range(N_CORES):
        out += res.results[c]["y"]
    return out.reshape(B, T, C)


if __name__ == "__main__":
    rng = np.random.default_rng(0)
    x = rng.standard_normal((B, T, C), dtype=np.float32)
    Wq = rng.standard_normal((C, H * D), dtype=np.float32) * 0.02
    Wk = rng.standard_normal((C, KV * D), dtype=np.float32) * 0.02
    Wv = rng.standard_normal((C, KV * D), dtype=np.float32) * 0.02
    Wo = rng.standard_normal((C, C), dtype=np.float32) * 0.02
    out = kernel(x=x, Wq=Wq, Wk=Wk, Wv=Wv, Wo=Wo)
    print("out", out.shape, out.dtype, float(np.abs(out).max()))


# revision 9
# speedup vs baseline: 7.6760x; 1.1781x over previous
"""GQA attention (B=2,T=2048,C=2048,H=16,KV=4,D=128) + RoPE + causal softmax
+ output projection, tensor-parallel over 8 NeuronCores (2 q-heads/core).

Contract: kernel(**inputs) takes full numpy inputs, returns full output.
Per-core partial outputs (o @ Wo[rows]) are summed on the host.
"""

import sys

sys.path.insert(0, "/opt/trn_rl_repo")

import numpy as np

import concourse.bacc as bacc
import concourse.mybir as mybir
import concourse.tile as tile
from concourse.bass_utils import run_bass_kernel_spmd

B, T, C = 2, 2048, 2048
H, KV, D = 16, 4, 128
G = H // KV
N_CORES = 8
HL = H // N_CORES  # 2 q-heads per core
BT = B * T  # 4096
NCH = BT // 512  # 8 token chunks of 512
KCH = C // 128  # 16 contraction chunks
QC = T // 512  # 4 q chunks per batch
KC = T // 128  # 16 k chunks per batch

F16 = mybir.dt.float16
F32 = mybir.dt.float32

_cache = {}


class _St:
    pass


def _emit_p1_chunk(nc, st, n):
    """QKV projections for one 512-token chunk."""
    mult = mybir.AluOpType.mult

    def rope(dst, ps, cos_s, sin_s, t0):
        t1 = st.rpool.tile([128, 512], F32, name="t1")
        t2 = st.rpool.tile([128, 512], F32, name="t2")
        nc.vector.tensor_tensor(t1[:], ps[:], cos_s[:, t0:t0 + 512], mult)
        nc.vector.tensor_tensor(t2[0:64, :], ps[64:128, :],
                                sin_s[0:64, t0:t0 + 512], mult)
        nc.vector.tensor_tensor(t2[64:128, :], ps[0:64, :],
                                sin_s[64:128, t0:t0 + 512], mult)
        nc.vector.tensor_tensor(dst, t1[:], t2[:], mybir.AluOpType.add)

    n0 = n * 512
    t0 = (n % QC) * 512  # rope-table offset (within batch)
    xt = st.xpool.tile([128, KCH, 512], F16, name="xt")
    dma_eng = nc.sync if n % 2 == 0 else nc.scalar
    dma_eng.dma_start(xt[:], st.xT_r[:, :, n0:n0 + 512])
    for h in range(HL):
        psq = st.psA.tile([128, 512], F32, name="psq", tag="psA")
        for kc in range(KCH):
            nc.tensor.matmul(
                psq[:],
                lhsT=st.wq_s[:, kc, h * D:(h + 1) * D],
                rhs=xt[:, kc, :],
                start=(kc == 0), stop=(kc == KCH - 1))
        rope(st.qT[:, h, n0:n0 + 512], psq, st.cosq_s, st.sinq_s, t0)
    psk = st.psA.tile([128, 512], F32, name="psk", tag="psA")
    for kc in range(KCH):
        nc.tensor.matmul(psk[:], lhsT=st.wk_s[:, kc, :],
                         rhs=xt[:, kc, :],
                         start=(kc == 0), stop=(kc == KCH - 1))
    rope(st.kT[:, n0:n0 + 512], psk, st.cosk_s, st.sink_s, t0)
    for s in range(4):
        psv = st.psA.tile([128, 512], F32, name="psv", tag="psA")
        for kc in range(KCH):
            nc.tensor.matmul(
                psv[:, 0:D],
                lhsT=xt[:, kc, s * 128:(s + 1) * 128],
                rhs=st.wv_s[:, kc, :],
                start=(kc == 0), stop=(kc == KCH - 1))
        nc.vector.tensor_copy(st.vv[:, n * 4 + s, :], psv[:, 0:D])


def _emit_p2_qc(nc, st, b, h, qc):
    """Attention for one (batch, head, 512-wide q chunk)."""
    mult = mybir.AluOpType.mult
    base = b * T
    q0 = base + qc * 512
    nkc = 4 * (qc + 1)  # causal: k chunks 0..nkc-1
    npair = nkc // 2
    psv2 = st.psA.tile([128, 512], F32, name="psv2", tag="psA")
    psd = st.psA.tile([128, 512], F32, name="psd", tag="psA")
    pts = [None] * npair

    def emit_scores(pi):
        pss = st.psS.tile([128, 2, 512], F32, name="pss")
        pt = st.ppool.tile([128, 2, 512], F16, name="pt")
        for i in range(2):
            kc = pi * 2 + i
            nc.tensor.matmul(
                pss[:, i, :],
                lhsT=st.kT[:, base + kc * 128:base + (kc + 1) * 128],
                rhs=st.qT[:, h, q0:q0 + 512],
                start=True, stop=True)
        nc.scalar.activation(pt[:], pss[:], mybir.ActivationFunctionType.Exp)
        d0 = pi * 2 - 4 * qc  # delta of first kc in pair
        if d0 >= 0:  # straddle pair: causal mask
            nc.vector.tensor_tensor(
                pt[:], pt[:], st.masks_s[:, d0:d0 + 2, :], mult)
        pts[pi] = pt

    def emit_consume(pi):
        pt = pts[pi]
        for i in range(2):
            kc = pi * 2 + i
            first = (kc == 0)
            last = (kc == nkc - 1)
            nc.tensor.matmul(
                psd[0:1, :], lhsT=st.ones_s[:],
                rhs=pt[:, i, :], start=first, stop=last)
            nc.tensor.matmul(
                psv2[:], lhsT=st.vv[:, b * KC + kc, :],
                rhs=pt[:, i, :], start=first, stop=last)

    # 1-stage software pipeline: PE's consume matmuls never sit
    # immediately behind the exp they wait on
    for pi in range(npair + 1):
        if pi < npair:
            emit_scores(pi)
        if pi > 0:
            emit_consume(pi - 1)

    recip = st.rpool.tile([1, 512], F32, name="recip")
    nc.vector.reciprocal(recip[:], psd[0:1, :])
    bc = st.bpool.tile([128, 512], F32, name="bc")
    nc.gpsimd.partition_broadcast(bc[:], recip[:])
    nc.vector.tensor_tensor(st.oT[:, h, q0:q0 + 512], psv2[:], bc[:], mult)


def _emit_p3_group(nc, st, b, qc):
    """Output projection for the 4 token tiles covered by (b, qc)."""
    for i in range(4):
        t0 = b * T + qc * 512 + i * 128
        for cc in range(C // 512):
            c0 = cc * 512
            pso = st.psA.tile([128, 512], F32, name="pso", tag="psA")
            for h in range(HL):
                nc.tensor.matmul(
                    pso[:], lhsT=st.oT[:, h, t0:t0 + 128],
                    rhs=st.wo_s[:, h, c0:c0 + 512],
                    start=(h == 0), stop=(h == HL - 1))
            yt = st.ypool.tile([128, 512], F16, name="yt")
            # balance PSUM eviction between ACT and DVE
            if st.p3_idx % 2 == 0:
                nc.scalar.copy(yt[:], pso[:])
            else:
                nc.vector.tensor_copy(yt[:], pso[:])
            dma_eng = nc.sync if st.p3_idx % 2 == 0 else nc.scalar
            st.p3_idx += 1
            dma_eng.dma_start(st.y[t0:t0 + 128, c0:c0 + 512], yt[:])


def _emit_body(nc, st):
    """One full forward pass, phases interleaved so ACT's exp work overlaps
    PE-heavy projection / output-projection stretches."""
    st.p3_idx = 0
    for n in range(QC):  # batch-0 token chunks
        _emit_p1_chunk(nc, st, n)
    for qc in range(QC):  # b0/h0 attention overlaps b1 projections
        _emit_p2_qc(nc, st, 0, 0, qc)
        _emit_p1_chunk(nc, st, QC + qc)
    for qc in range(QC):  # b0/h1 attention overlaps b0 output projection
        _emit_p2_qc(nc, st, 0, 1, qc)
        _emit_p3_group(nc, st, 0, qc)
    for qc in range(QC):  # b1 attention (both heads) + b1 output projection
        _emit_p2_qc(nc, st, 1, 0, qc)
        _emit_p2_qc(nc, st, 1, 1, qc)
        _emit_p3_group(nc, st, 1, qc)


def _build_program(reps=1, loop_n=None):
    nc = bacc.Bacc("TRN2", target_bir_lowering=False, debug=False,
                   num_devices=N_CORES)

    xT = nc.dram_tensor("xT", [C, BT], F16, kind="ExternalInput").ap()
    wq = nc.dram_tensor("wq", [C, HL * D], F16, kind="ExternalInput").ap()
    wk = nc.dram_tensor("wk", [C, D], F16, kind="ExternalInput").ap()
    wv = nc.dram_tensor("wv", [C, D], F16, kind="ExternalInput").ap()
    wo = nc.dram_tensor("wo", [HL * D, C], F16, kind="ExternalInput").ap()
    cosq = nc.dram_tensor("cosq", [D, T], F32, kind="ExternalInput").ap()
    sinq = nc.dram_tensor("sinq", [D, T], F32, kind="ExternalInput").ap()
    cosk = nc.dram_tensor("cosk", [D, T], F32, kind="ExternalInput").ap()
    sink = nc.dram_tensor("sink", [D, T], F32, kind="ExternalInput").ap()
    masks = nc.dram_tensor("masks", [128, 4, 512], F16, kind="ExternalInput").ap()
    y = nc.dram_tensor("y", [BT, C], F16, kind="ExternalOutput").ap()

    st = _St()
    with tile.TileContext(nc) as tc:
        with (
            tc.tile_pool(name="cpool", bufs=1) as cpool,
            tc.tile_pool(name="xpool", bufs=2) as xpool,
            tc.tile_pool(name="ppool", bufs=8) as ppool,
            tc.tile_pool(name="rpool", bufs=3) as rpool,
            tc.tile_pool(name="bpool", bufs=2) as bpool,
            tc.tile_pool(name="ypool", bufs=8) as ypool,
            tc.tile_pool(name="psA", bufs=4, space="PSUM") as psA,
            tc.tile_pool(name="psS", bufs=2, space="PSUM") as psS,
        ):
            st.xpool, st.ppool, st.rpool, st.bpool, st.ypool = (
                xpool, ppool, rpool, bpool, ypool)
            st.psA, st.psS = psA, psS

            # ---- persistent SBUF state ----
            st.wq_s = cpool.tile([128, KCH, HL * D], F16, name="wq_s")
            st.wk_s = cpool.tile([128, KCH, D], F16, name="wk_s")
            st.wv_s = cpool.tile([128, KCH, D], F16, name="wv_s")
            st.wo_s = cpool.tile([128, HL, C], F16, name="wo_s")
            st.cosq_s = cpool.tile([D, T], F32, name="cosq_s")
            st.sinq_s = cpool.tile([D, T], F32, name="sinq_s")
            st.cosk_s = cpool.tile([D, T], F32, name="cosk_s")
            st.sink_s = cpool.tile([D, T], F32, name="sink_s")
            st.masks_s = cpool.tile([128, 4, 512], F16, name="masks_s")
            st.ones_s = cpool.tile([128, 1], F16, name="ones_s")
            st.qT = cpool.tile([D, HL, BT], F16, name="qT")
            st.kT = cpool.tile([D, BT], F16, name="kT")
            st.vv = cpool.tile([128, BT // 128, D], F16, name="vv")
            st.oT = cpool.tile([D, HL, BT], F16, name="oT")

            nc.sync.dma_start(st.wq_s[:], wq.rearrange("(k p) m -> p k m", p=128))
            nc.sync.dma_start(st.wk_s[:], wk.rearrange("(k p) m -> p k m", p=128))
            nc.sync.dma_start(st.wv_s[:], wv.rearrange("(k p) m -> p k m", p=128))
            nc.scalar.dma_start(st.wo_s[:], wo.rearrange("(r p) n -> p r n", p=128))
            nc.scalar.dma_start(st.cosq_s[:], cosq[:])
            nc.scalar.dma_start(st.sinq_s[:], sinq[:])
            nc.scalar.dma_start(st.cosk_s[:], cosk[:])
            nc.scalar.dma_start(st.sink_s[:], sink[:])
            nc.scalar.dma_start(st.masks_s[:], masks[:])
            nc.vector.memset(st.ones_s[:], 1.0)

            st.xT_r = xT.rearrange("(k p) n -> p k n", p=128)
            st.y = y

            if loop_n is not None:
                with tc.For_i(0, loop_n, 1):
                    _emit_body(nc, st)
            else:
                for _rep in range(reps):
                    _emit_body(nc, st)

    nc.compile()
    return nc


def _rope_tables():
    inv = (1.0 / (10000.0 ** (np.arange(0, D, 2, dtype=np.float32) / D)))
    f = np.arange(T, dtype=np.float32)[:, None] * inv[None, :]  # [T, 64]
    cos = np.concatenate([np.cos(f)] * 2, axis=-1).astype(np.float32)  # [T,128]
    sin = np.concatenate([np.sin(f)] * 2, axis=-1).astype(np.float32)
    sgn = np.ones((D,), np.float32)
    sgn[0:64] = -1.0  # sign-folded for the rotate-half formulation
    sinf = sin * sgn[None, :]
    alpha = np.float32(1.0 / np.sqrt(D))
    return (cos.T.copy() * alpha, sinf.T.copy() * alpha,
            cos.T.copy(), sinf.T.copy())


def _masks():
    i = np.arange(128)[:, None]
    j = np.arange(512)[None, :]
    m = np.stack([(j >= i + 128 * d) for d in range(4)], axis=0)  # [4,128,512]
    return np.ascontiguousarray(m.transpose(1, 0, 2)).astype(np.float16)


def make_in_maps(x, Wq, Wk, Wv, Wo):
    xT = np.ascontiguousarray(
        x.reshape(BT, C).T).astype(np.float16)  # [C, BT]
    cq, sq, ck, sk = _rope_tables()
    mk = _masks()
    in_maps = []
    for c in range(N_CORES):
        g = c // 2  # kv head for this core's 2 q-heads
        in_maps.append({
            "xT": xT,
            "wq": np.ascontiguousarray(
                Wq[:, c * HL * D:(c + 1) * HL * D]).astype(np.float16),
            "wk": np.ascontiguousarray(
                Wk[:, g * D:(g + 1) * D]).astype(np.float16),
            "wv": np.ascontiguousarray(
                Wv[:, g * D:(g + 1) * D]).astype(np.float16),
            "wo": np.ascontiguousarray(
                Wo[c * HL * D:(c + 1) * HL * D, :]).astype(np.float16),
            "cosq": cq, "sinq": sq, "cosk": ck, "sink": sk,
            "masks": mk,
        })
    return in_maps


def get_program(reps=1, loop_n=None):
    key = ("nc", reps, loop_n)
    if key not in _cache:
        _cache[key] = _build_program(reps, loop_n)
    return _cache[key]


def kernel(x, Wq, Wk, Wv, Wo):
    nc = get_program()
    in_maps = make_in_maps(x, Wq, Wk, Wv, Wo)
    res = run_bass_kernel_spmd(nc, in_maps, core_ids=list(range(N_CORES)))
    out = np.zeros((BT, C), np.float32)
    for c in range(N_CORES):
        out += res.results[c]["y"].astype(np.float32)
    return out.reshape(B, T, C)


if __name__ == "__main__":
    rng = np.random.default_rng(0)
    x = rng.standard_normal((B, T, C), dtype=np.float32)
    Wq = rng.standard_normal((C, H * D), dtype=np.float32) * 0.02
    Wk = rng.standard_normal((C, KV * D), dtype=np.float32) * 0.02
    Wv = rng.standard_normal((C, KV * D), dtype=np.float32) * 0.02
    Wo = rng.standard_normal((C, C), dtype=np.float32) * 0.02
    out = kernel(x=x, Wq=Wq, Wk=Wk, Wv=Wv, Wo=Wo)
    print("out", out.shape, out.dtype, float(np.abs(out).max()))
